# revision 24
# baseline (speedup 1.0000x reference)
"""CapsLayer kernel v7: j-sharded, all-fp8-e3m4 stream (W s=32, x s=2).

Math: the reference's routing loop is dead (softmax over a size-1 axis is
identically 1), so the output is
    s[b, j, l] = sum_{i,k} W[i, j, l, k] * inputs[b, i, k]
    vj = squash(s, axis=l)  ->  [B, 1, NUM_CAPS, DIM_CAPS]

Sharding: W splits over NUM_CAPS j (4 capsules / 128 output columns per
core); inputs are replicated.  Everything stays on-core — no collectives.

v7 vs v5/v6: both W and x stream as fp8 e3m4 (W pre-scaled by 32, x by 2
with clipping at the e3m4 max), cutting the stream to 5.25 MB/core.
Measured rel err 1.71e-2 vs the 2e-2 gate (HW matches the numpy
quantization sim to 4 digits on previous versions).  The combined 1/64
dequant folds into the ACT Square scale and the fsc fused op.  Dual-queue
DMA (v6) is reverted — the 16 shared DMA engines are HBM-limited, and
split queues only de-ordered the tail — but v6's fused
scalar_tensor_tensor epilogue and the k-major two-DMA tile 15 stay.

PE: 16 accumulation chains (one per k), round-robin over the four
32-column PE array groups via tile_position; chain g accumulates into
PSUM partitions [32g, 32g+32).  A final 128x32 fp32 identity-block
matmul folds the partials into s[b, n].  At fp8 the kernel is jointly
PE/DMA-limited: W's 4.2M elements enter the 128-row array at 1 col/cycle
(13.6 us floor) while the stream takes ~14 us.

Raw Bass: this walrus build rejects instructions carrying 2+ sem waits, so
all sync is standalone wait_ge ops.  DVE/ACT same-engine RAW needs explicit
semaphores (the pipelines do not interlock through SBUF).
"""

from contextlib import ExitStack

import numpy as np
import ml_dtypes

B = 32
IN_CAPS = 2048
IN_DIM = 16
NUM_CAPS = 32
DIM_CAPS = 32
NCORES = 8
JPC = NUM_CAPS // NCORES          # 4 capsules per core
NJL = JPC * DIM_CAPS              # 128 output columns per core
P = 128
NTILES = IN_CAPS // P             # 16
X_E3 = True                       # False -> x streams as bf16 (safer margin)
XB = IN_DIM * B * (1 if X_E3 else 2)
WB = NJL * IN_DIM                 # 2048 B of e3m4 w per tile-row
TB = XB + WB                      # bytes per tile-row
NQUAD = 4                         # 4 quad-tile DMAs (10240B rows)
SBB = NTILES * TB                 # SBUF bytes per partition
NG = 4                            # PE column groups
KH = IN_DIM // 2                  # 8 k's per half of tile 15
EPS = 1e-7
WSCALE = 32.0                     # W premultiplier before e3m4 cast
XSCALE = 2.0 if X_E3 else 1.0     # x premultiplier before e3m4 cast
SCALE = WSCALE * XSCALE

_CACHE = {}


def _build():
    import concourse.bass as bass
    from concourse import mybir

    f32 = mybir.dt.float32
    xdt = mybir.dt.float8e3 if X_E3 else mybir.dt.bfloat16
    e3 = mybir.dt.float8e3
    u8 = mybir.dt.uint8
    nc = bass.Bass()
    xwq = nc.declare_dram_parameter("xwq", [NQUAD * P, 4 * TB], u8, isOutput=False)
    ed = nc.declare_dram_parameter("ed", [P, B], f32, isOutput=False)
    out = nc.declare_dram_parameter("out", [B, NJL], f32, isOutput=True)

    with ExitStack() as ctx:
        xw_sb = ctx.enter_context(nc.sbuf_tensor([P, SBB], u8))
        e_sb = ctx.enter_context(nc.sbuf_tensor([P, B], f32))
        p4_sb = ctx.enter_context(nc.sbuf_tensor([P, NJL], f32))
        sq = ctx.enter_context(nc.sbuf_tensor([B, NJL], f32))
        ss = ctx.enter_context(nc.sbuf_tensor([B, JPC], f32))
        rt = ctx.enter_context(nc.sbuf_tensor([B, JPC], f32))
        den = ctx.enter_context(nc.sbuf_tensor([B, JPC], f32))
        rden = ctx.enter_context(nc.sbuf_tensor([B, JPC], f32))
        fsc = ctx.enter_context(nc.sbuf_tensor([B, JPC], f32))
        epst = ctx.enter_context(nc.sbuf_tensor([B, 1], f32))
        warm = ctx.enter_context(nc.sbuf_tensor([B, 1], f32))
        vout = ctx.enter_context(nc.sbuf_tensor([B, NJL], f32))
        ps4 = ctx.enter_context(nc.psum_tensor([P, NJL], f32))
        pss = ctx.enter_context(nc.psum_tensor([B, NJL], f32))

        # chunk sems: one per quad
        tsem = [ctx.enter_context(nc.semaphore(f"t{c}")) for c in range(NQUAD)]
        esem = ctx.enter_context(nc.semaphore("esem"))
        pe_sem = ctx.enter_context(nc.semaphore("pe"))
        act_sem = ctx.enter_context(nc.semaphore("act"))
        dve_sem = ctx.enter_context(nc.semaphore("dve"))
        odma = ctx.enter_context(nc.semaphore("odma"))
        block = ctx.enter_context(nc.Block())

        @block.sync
        def _(sync):
            for c in range(NQUAD):
                sync.dma_start(
                    out=xw_sb[:, c * 4 * TB:(c + 1) * 4 * TB],
                    in_=xwq[c * P:(c + 1) * P, :],
                ).then_inc(tsem[c], 16)
            sync.wait_ge(dve_sem, 6)
            sync.dma_start(out=out[:, :], in_=vout[:, :]).then_inc(odma, 16)
            sync.wait_ge(odma, 16)

        @block.tensor
        def _(tensor):
            def tile_mms(base, ti, kmajor, krange):
                xview = xw_sb[:, base:base + XB].bitcast(xdt)
                wv = xw_sb[:, base + XB:base + TB].bitcast(e3)
                if kmajor:
                    wview = wv.rearrange("p (k n) -> p k n", n=NJL)
                else:
                    wview = wv.rearrange("p (n k) -> p n k", k=IN_DIM)
                mm = None
                for k in krange:
                    g = k % NG
                    rhs = wview[:, k, :] if kmajor else wview[:, :, k]
                    mm = nc.tensor.matmul(
                        ps4[32 * g:32 * (g + 1), :],
                        xview[:, k * B:(k + 1) * B],
                        rhs,
                        start=(ti == 0 and k < NG),
                        stop=(ti == NTILES - 1 and k >= IN_DIM - NG),
                        tile_position=(0, 32 * g),
                    )
                return mm

            for c in range(NQUAD):
                tensor.wait_ge(tsem[c], 16)
                for h in range(4):
                    mm = tile_mms((4 * c + h) * TB, 4 * c + h, False, range(IN_DIM))
            mm.then_inc(pe_sem, 1)
            # merge the 4 partial chains: s[b, n] = sum_g p4[32g+b, n]
            tensor.wait_ge(dve_sem, 1)
            tensor.wait_ge(esem, 16)
            nc.tensor.matmul(
                pss[:, :],
                e_sb[:, :],
                p4_sb[:, :],
                start=True,
                stop=True,
            ).then_inc(pe_sem, 1)

        @block.vector
        def _(vector):
            nc.vector.memset(epst[:, :], EPS)
            vector.wait_ge(pe_sem, 1)
            nc.vector.tensor_copy(p4_sb[:, :], ps4[:, :]).then_inc(dve_sem, 1)
            vector.wait_ge(act_sem, 1)
            red = nc.vector.reduce_sum(
                out=ss[:, :],
                in_=sq[:, :].rearrange("p (g d) -> p g d", g=JPC),
                axis=mybir.AxisListType.X,
            )
            red.then_inc(dve_sem, 1)
            vector.wait_ge(act_sem, 2)
            # den = (ss + 1) * rt
            nc.vector.scalar_tensor_tensor(
                out=den[:, :], in0=ss[:, :], scalar=1.0, in1=rt[:, :],
                op0=mybir.AluOpType.add, op1=mybir.AluOpType.mult,
            ).then_inc(dve_sem, 1)
            vector.wait_ge(dve_sem, 3)
            nc.vector.reciprocal(out=rden[:, :], in_=den[:, :]).then_inc(dve_sem, 1)
            vector.wait_ge(dve_sem, 4)
            # fsc = (ss / SCALE) * rden ; cancels the SCALE inside pss below
            nc.vector.scalar_tensor_tensor(
                out=fsc[:, :], in0=ss[:, :], scalar=1.0 / SCALE, in1=rden[:, :],
                op0=mybir.AluOpType.mult, op1=mybir.AluOpType.mult,
            ).then_inc(dve_sem, 1)
            vector.wait_ge(dve_sem, 5)
            nc.vector.tensor_mul(
                vout[:, :].rearrange("p (g d) -> p g d", g=JPC),
                pss[:, :].rearrange("p (g d) -> p g d", g=JPC),
                fsc[:, :].unsqueeze(2).broadcast_to([B, JPC, DIM_CAPS]),
            ).then_inc(dve_sem, 1)

        @block.scalar
        def _(scalar):
            scalar.dma_start(out=e_sb[:, :], in_=ed[:, :]).then_inc(esem, 16)
            # dummy Sqrt pulls the ~1.3us ACT table load off the epilogue
            # critical path; Square/Sqrt share the sqrt_and_others table
            nc.scalar.activation(
                out=warm[:, :], in_=warm[:, :],
                func=mybir.ActivationFunctionType.Sqrt, bias=warm[:, :],
            )
            # sq = (pss/SCALE)^2 = s^2 straight out of PSUM
            scalar.wait_ge(pe_sem, 2)
            nc.scalar.activation(
                out=sq[:, :], in_=pss[:, :],
                func=mybir.ActivationFunctionType.Square, bias=0.0,
                scale=1.0 / SCALE,
            ).then_inc(act_sem, 1)
            scalar.wait_ge(dve_sem, 2)
            nc.scalar.activation(
                out=rt[:, :], in_=ss[:, :],
                func=mybir.ActivationFunctionType.Sqrt, bias=epst[:, :],
            ).then_inc(act_sem, 1)

    return nc


def _in_maps(inputs, W):
    e3 = ml_dtypes.float8_e3m4
    x_t = np.ascontiguousarray(
        np.transpose(inputs, (1, 2, 0)).reshape(IN_CAPS, IN_DIM * B)
    )                                                             # [i, (k, b)]
    if X_E3:
        lim = float(ml_dtypes.finfo(e3).max)
        x_q = np.clip(x_t * XSCALE, -lim, lim).astype(e3)
    else:
        x_q = x_t.astype(ml_dtypes.bfloat16)
    xbytes = x_q.view(np.uint8).reshape(NTILES, P, XB)
    erow = np.zeros((P, B), dtype=np.float32)
    erow[np.arange(P), np.arange(P) % B] = 1.0                    # E[p%32 == b]
    maps = []
    for c in range(NCORES):
        w_q = (W[:, c * JPC:(c + 1) * JPC] * WSCALE).astype(e3)   # [i, j, l, k]
        wnk = w_q.reshape(IN_CAPS, WB).view(np.uint8).reshape(NTILES, P, WB)
        rows = np.empty((NTILES, P, TB), dtype=np.uint8)
        rows[:, :, :XB] = xbytes
        rows[:, :, XB:] = wnk
        xwqc = (
            rows
            .reshape(NQUAD, 4, P, TB)
            .transpose(0, 2, 1, 3)
            .reshape(NQUAD * P, 4 * TB)
        )
        maps.append({"xwq": np.ascontiguousarray(xwqc), "ed": erow})
    return maps


def kernel(inputs, W):
    from concourse.bass_utils import run_bass_kernel_spmd

    inputs = np.asarray(inputs, dtype=np.float32)
    W = np.asarray(W, dtype=np.float32)
    if "nc" not in _CACHE:
        _CACHE["nc"] = _build()
    res = run_bass_kernel_spmd(_CACHE["nc"], _in_maps(inputs, W), list(range(NCORES)))
    return np.concatenate(
        [res.results[c]["out"].reshape(B, 1, JPC, DIM_CAPS) for c in range(NCORES)],
        axis=2,
    )


# revision 30
# speedup vs baseline: 1.0116x; 1.0116x over previous
"""CapsLayer kernel v7: j-sharded, all-fp8-e3m4 stream (W s=32, x s=2).

Math: the reference's routing loop is dead (softmax over a size-1 axis is
identically 1), so the output is
    s[b, j, l] = sum_{i,k} W[i, j, l, k] * inputs[b, i, k]
    vj = squash(s, axis=l)  ->  [B, 1, NUM_CAPS, DIM_CAPS]

Sharding: W splits over NUM_CAPS j (4 capsules / 128 output columns per
core); inputs are replicated.  Everything stays on-core — no collectives.

v7 vs v5/v6: both W and x stream as fp8 e3m4 (W pre-scaled by 32, x by 2
with clipping at the e3m4 max), cutting the stream to 5.25 MB/core.
Measured rel err 1.71e-2 vs the 2e-2 gate (HW matches the numpy
quantization sim to 4 digits on previous versions).  The combined 1/64
dequant folds into the ACT Square scale and the fsc fused op.  Dual-queue
DMA (v6) is reverted — the 16 shared DMA engines are HBM-limited, and
split queues only de-ordered the tail — but v6's fused
scalar_tensor_tensor epilogue and the k-major two-DMA tile 15 stay.

PE: 16 accumulation chains (one per k), round-robin over the four
32-column PE array groups via tile_position; chain g accumulates into
PSUM partitions [32g, 32g+32).  A final 128x32 fp32 identity-block
matmul folds the partials into s[b, n].  At fp8 the kernel is jointly
PE/DMA-limited: W's 4.2M elements enter the 128-row array at 1 col/cycle
(13.6 us floor) while the stream takes ~14 us.

Raw Bass: this walrus build rejects instructions carrying 2+ sem waits, so
all sync is standalone wait_ge ops.  DVE/ACT same-engine RAW needs explicit
semaphores (the pipelines do not interlock through SBUF).
"""

from contextlib import ExitStack

import numpy as np
import ml_dtypes

B = 32
IN_CAPS = 2048
IN_DIM = 16
NUM_CAPS = 32
DIM_CAPS = 32
NCORES = 8
JPC = NUM_CAPS // NCORES          # 4 capsules per core
NJL = JPC * DIM_CAPS              # 128 output columns per core
P = 128
NTILES = IN_CAPS // P             # 16
X_E3 = True                       # False -> x streams as bf16 (safer margin)
XB = IN_DIM * B * (1 if X_E3 else 2)
WB = NJL * IN_DIM                 # 2048 B of e3m4 w per tile-row
TB = XB + WB                      # bytes per tile-row
NQUAD = 3                         # 3 quad-tile DMAs (tiles 0..11, 10240B rows)
SBB = NTILES * TB                 # SBUF bytes per partition
NG = 4                            # PE column groups
KH = IN_DIM // 2                  # 8 k's per half of tile 15
EPS = 1e-7
WSCALE = 32.0                     # W premultiplier before e3m4 cast
XSCALE = 2.0 if X_E3 else 1.0     # x premultiplier before e3m4 cast
SCALE = WSCALE * XSCALE

_CACHE = {}


def _build():
    import concourse.bass as bass
    from concourse import mybir

    f32 = mybir.dt.float32
    xdt = mybir.dt.float8e3 if X_E3 else mybir.dt.bfloat16
    e3 = mybir.dt.float8e3
    u8 = mybir.dt.uint8
    nc = bass.Bass()
    xwq = nc.declare_dram_parameter("xwq", [NQUAD * P, 4 * TB], u8, isOutput=False)
    xwp = nc.declare_dram_parameter("xwp", [P, 2 * TB], u8, isOutput=False)
    xws = nc.declare_dram_parameter("xws", [2 * P, TB], u8, isOutput=False)
    ed = nc.declare_dram_parameter("ed", [P, B], f32, isOutput=False)
    out = nc.declare_dram_parameter("out", [B, NJL], f32, isOutput=True)

    with ExitStack() as ctx:
        xw_sb = ctx.enter_context(nc.sbuf_tensor([P, SBB], u8))
        e_sb = ctx.enter_context(nc.sbuf_tensor([P, B], f32))
        p4_sb = ctx.enter_context(nc.sbuf_tensor([P, NJL], f32))
        sq = ctx.enter_context(nc.sbuf_tensor([B, NJL], f32))
        ss = ctx.enter_context(nc.sbuf_tensor([B, JPC], f32))
        rt = ctx.enter_context(nc.sbuf_tensor([B, JPC], f32))
        den = ctx.enter_context(nc.sbuf_tensor([B, JPC], f32))
        rden = ctx.enter_context(nc.sbuf_tensor([B, JPC], f32))
        fsc = ctx.enter_context(nc.sbuf_tensor([B, JPC], f32))
        epst = ctx.enter_context(nc.sbuf_tensor([B, 1], f32))
        warm = ctx.enter_context(nc.sbuf_tensor([B, 1], f32))
        vout = ctx.enter_context(nc.sbuf_tensor([B, NJL], f32))
        ps4 = ctx.enter_context(nc.psum_tensor([P, NJL], f32))
        pss = ctx.enter_context(nc.psum_tensor([B, NJL], f32))

        # chunk sems: quads 0..2, pair 12-13, single 14, tile 15 halves A/B
        tsem = [ctx.enter_context(nc.semaphore(f"t{c}")) for c in range(7)]
        esem = ctx.enter_context(nc.semaphore("esem"))
        pe_sem = ctx.enter_context(nc.semaphore("pe"))
        act_sem = ctx.enter_context(nc.semaphore("act"))
        dve_sem = ctx.enter_context(nc.semaphore("dve"))
        odma = ctx.enter_context(nc.semaphore("odma"))
        block = ctx.enter_context(nc.Block())

        B12 = 12 * TB                     # sbuf byte base of tile 12
        B14 = 14 * TB                     # sbuf byte base of tile 14
        B15 = 15 * TB                     # sbuf byte base of tile 15
        HALF_A = XB + KH * NJL            # tile-15 first-DMA bytes (x + k<8)

        @block.sync
        def _(sync):
            for c in range(NQUAD):
                sync.dma_start(
                    out=xw_sb[:, c * 4 * TB:(c + 1) * 4 * TB],
                    in_=xwq[c * P:(c + 1) * P, :],
                ).then_inc(tsem[c], 16)
            sync.dma_start(
                out=xw_sb[:, B12:B12 + 2 * TB], in_=xwp[:, :],
            ).then_inc(tsem[3], 16)
            sync.dma_start(
                out=xw_sb[:, B14:B14 + TB], in_=xws[0:P, :],
            ).then_inc(tsem[4], 16)
            sync.dma_start(
                out=xw_sb[:, B15:B15 + HALF_A], in_=xws[P:2 * P, :HALF_A],
            ).then_inc(tsem[5], 16)
            sync.dma_start(
                out=xw_sb[:, B15 + HALF_A:B15 + TB], in_=xws[P:2 * P, HALF_A:],
            ).then_inc(tsem[6], 16)
            sync.wait_ge(dve_sem, 6)
            sync.dma_start(out=out[:, :], in_=vout[:, :]).then_inc(odma, 16)
            sync.wait_ge(odma, 16)

        @block.tensor
        def _(tensor):
            def tile_mms(base, ti, kmajor, krange):
                xview = xw_sb[:, base:base + XB].bitcast(xdt)
                wv = xw_sb[:, base + XB:base + TB].bitcast(e3)
                if kmajor:
                    wview = wv.rearrange("p (k n) -> p k n", n=NJL)
                else:
                    wview = wv.rearrange("p (n k) -> p n k", k=IN_DIM)
                mm = None
                for k in krange:
                    g = k % NG
                    rhs = wview[:, k, :] if kmajor else wview[:, :, k]
                    mm = nc.tensor.matmul(
                        ps4[32 * g:32 * (g + 1), :],
                        xview[:, k * B:(k + 1) * B],
                        rhs,
                        start=(ti == 0 and k < NG),
                        stop=(ti == NTILES - 1 and k >= IN_DIM - NG),
                        tile_position=(0, 32 * g),
                    )
                return mm

            for c in range(NQUAD):
                tensor.wait_ge(tsem[c], 16)
                for h in range(4):
                    tile_mms((4 * c + h) * TB, 4 * c + h, False, range(IN_DIM))
            tensor.wait_ge(tsem[3], 16)
            tile_mms(B12, 12, False, range(IN_DIM))
            tile_mms(B12 + TB, 13, False, range(IN_DIM))
            tensor.wait_ge(tsem[4], 16)
            tile_mms(B14, 14, False, range(IN_DIM))
            tensor.wait_ge(tsem[5], 16)
            tile_mms(B15, 15, True, range(KH))
            tensor.wait_ge(tsem[6], 16)
            mm = tile_mms(B15, 15, True, range(KH, IN_DIM))
            mm.then_inc(pe_sem, 1)
            # merge the 4 partial chains: s[b, n] = sum_g p4[32g+b, n]
            tensor.wait_ge(dve_sem, 1)
            tensor.wait_ge(esem, 16)
            nc.tensor.matmul(
                pss[:, :],
                e_sb[:, :],
                p4_sb[:, :],
                start=True,
                stop=True,
            ).then_inc(pe_sem, 1)

        @block.vector
        def _(vector):
            nc.vector.memset(epst[:, :], EPS)
            vector.wait_ge(pe_sem, 1)
            nc.vector.tensor_copy(p4_sb[:, :], ps4[:, :]).then_inc(dve_sem, 1)
            vector.wait_ge(act_sem, 1)
            red = nc.vector.reduce_sum(
                out=ss[:, :],
                in_=sq[:, :].rearrange("p (g d) -> p g d", g=JPC),
                axis=mybir.AxisListType.X,
            )
            red.then_inc(dve_sem, 1)
            vector.wait_ge(act_sem, 2)
            # den = (ss + 1) * rt
            nc.vector.scalar_tensor_tensor(
                out=den[:, :], in0=ss[:, :], scalar=1.0, in1=rt[:, :],
                op0=mybir.AluOpType.add, op1=mybir.AluOpType.mult,
            ).then_inc(dve_sem, 1)
            vector.wait_ge(dve_sem, 3)
            nc.vector.reciprocal(out=rden[:, :], in_=den[:, :]).then_inc(dve_sem, 1)
            vector.wait_ge(dve_sem, 4)
            # fsc = (ss / SCALE) * rden ; cancels the SCALE inside pss below
            nc.vector.scalar_tensor_tensor(
                out=fsc[:, :], in0=ss[:, :], scalar=1.0 / SCALE, in1=rden[:, :],
                op0=mybir.AluOpType.mult, op1=mybir.AluOpType.mult,
            ).then_inc(dve_sem, 1)
            vector.wait_ge(dve_sem, 5)
            nc.vector.tensor_mul(
                vout[:, :].rearrange("p (g d) -> p g d", g=JPC),
                pss[:, :].rearrange("p (g d) -> p g d", g=JPC),
                fsc[:, :].unsqueeze(2).broadcast_to([B, JPC, DIM_CAPS]),
            ).then_inc(dve_sem, 1)

        @block.scalar
        def _(scalar):
            scalar.dma_start(out=e_sb[:, :], in_=ed[:, :]).then_inc(esem, 16)
            # dummy Sqrt pulls the ~1.3us ACT table load off the epilogue
            # critical path; Square/Sqrt share the sqrt_and_others table
            nc.scalar.activation(
                out=warm[:, :], in_=warm[:, :],
                func=mybir.ActivationFunctionType.Sqrt, bias=warm[:, :],
            )
            # sq = (pss/SCALE)^2 = s^2 straight out of PSUM
            scalar.wait_ge(pe_sem, 2)
            nc.scalar.activation(
                out=sq[:, :], in_=pss[:, :],
                func=mybir.ActivationFunctionType.Square, bias=0.0,
                scale=1.0 / SCALE,
            ).then_inc(act_sem, 1)
            scalar.wait_ge(dve_sem, 2)
            nc.scalar.activation(
                out=rt[:, :], in_=ss[:, :],
                func=mybir.ActivationFunctionType.Sqrt, bias=epst[:, :],
            ).then_inc(act_sem, 1)

    return nc


def _in_maps(inputs, W):
    e3 = ml_dtypes.float8_e3m4
    x_t = np.ascontiguousarray(
        np.transpose(inputs, (1, 2, 0)).reshape(IN_CAPS, IN_DIM * B)
    )                                                             # [i, (k, b)]
    if X_E3:
        lim = float(ml_dtypes.finfo(e3).max)
        x_q = np.clip(x_t * XSCALE, -lim, lim).astype(e3)
    else:
        x_q = x_t.astype(ml_dtypes.bfloat16)
    xbytes = x_q.view(np.uint8).reshape(NTILES, P, XB)
    erow = np.zeros((P, B), dtype=np.float32)
    erow[np.arange(P), np.arange(P) % B] = 1.0                    # E[p%32 == b]
    maps = []
    for c in range(NCORES):
        w_q = (W[:, c * JPC:(c + 1) * JPC] * WSCALE).astype(e3)   # [i, j, l, k]
        wnk = w_q.reshape(IN_CAPS, WB).view(np.uint8).reshape(NTILES, P, WB)
        rows = np.empty((NTILES, P, TB), dtype=np.uint8)
        rows[:, :, :XB] = xbytes
        rows[:, :, XB:] = wnk
        # tile 15 goes k-major so its W halves are contiguous byte ranges
        w15 = np.ascontiguousarray(
            np.transpose(w_q[15 * P:], (3, 1, 2, 0))              # [k, j, l, p]
            .reshape(IN_DIM * NJL, P).T                           # -> [p, (k n)]
        )
        rows[15, :, XB:] = w15.view(np.uint8)
        xwqc = (
            rows[:4 * NQUAD]
            .reshape(NQUAD, 4, P, TB)
            .transpose(0, 2, 1, 3)
            .reshape(NQUAD * P, 4 * TB)
        )
        xwpc = (
            rows[12:14]
            .reshape(1, 2, P, TB)
            .transpose(0, 2, 1, 3)
            .reshape(P, 2 * TB)
        )
        maps.append({
            "xwq": np.ascontiguousarray(xwqc),
            "xwp": np.ascontiguousarray(xwpc),
            "xws": rows[14:].reshape(2 * P, TB).copy(),
            "ed": erow,
        })
    return maps


def kernel(inputs, W):
    from concourse.bass_utils import run_bass_kernel_spmd

    inputs = np.asarray(inputs, dtype=np.float32)
    W = np.asarray(W, dtype=np.float32)
    if "nc" not in _CACHE:
        _CACHE["nc"] = _build()
    res = run_bass_kernel_spmd(_CACHE["nc"], _in_maps(inputs, W), list(range(NCORES)))
    return np.concatenate(
        [res.results[c]["out"].reshape(B, 1, JPC, DIM_CAPS) for c in range(NCORES)],
        axis=2,
    )
